# revision 26
# baseline (speedup 1.0000x reference)
"""Trainium2 Bass kernel for 16-head MHA (b=2, n=2048, c=1024, d=64).

Reference semantics (note the inverted scale "bug" reproduced faithfully):
    qkv = x @ W_qkv + b_qkv
    scores = (q @ k^T) * sqrt(d)          # multiplied, not divided
    out = softmax(scores) @ v
    y = concat_heads(out) @ W_proj + b_proj

Sharding: tensor-parallel over heads. Each of the 8 cores computes QKV +
attention for its 2 heads (head-dim-transposed layouts so no activation
transposes are needed beyond one x^T pass), then a single AllToAll moves the
per-head attention outputs into a row-sharded layout and each core computes
the final projection for its 512-row output shard. Host concatenates shards.

Precision strategy: exactness is needed only upstream of exp (the *sqrt(d)
score scale amplifies absolute errors into exp-space relative errors).
Those matmuls use an fp16 hi/lo split, 3 accumulating passes
(hi*hi + hi*lo + lo*hi; the dropped lo*lo term is negligible) — products
are exact in the PE and accumulate in fp32. The row-max pass needs no
precision at all (softmax renormalization cancels any max offset exactly),
so it runs single-pass on the hi parts. exp scores, A@V and the projection
are plain fp16 (inputs exact to ~1e-3, no amplification).

Softmax plumbing: the transposed-score matmul gets an extra contraction row
(ones in k^T_ext, -rowmax in q_ext) so exp(psum) needs no separate bias op;
V gets an extra ones *column* so the A@V matmul also yields the softmax
denominators; one reciprocal + gpsimd partition-broadcast + multiply
normalizes each head-output block (emitted late so the DVE reciprocal never
blocks the PE-feeding copy chain).

The attention phase is software-pipelined at emission level: the row-max
pass of stage i+1 is interleaved with the transposed pass of stage i, so
the PE never drains while the stats chain (transpose/flatten/DMA) of the
next stage resolves.
"""

import sys
from contextlib import ExitStack

sys.path.insert(0, "/opt/trn_rl_repo")

import numpy as np

import concourse.bass as bass
import concourse.tile as tile
from concourse import bacc, mybir
from concourse import bass_utils
from concourse.masks import make_identity

# Problem shape (hardcoded per contract)
B, N, C = 2, 2048, 1024
H, D = 16, 64
NCORES = 8
HPC = H // NCORES          # heads per core = 2
D2 = HPC * D               # 128 = per-core slice of the concat dim
R = B * N                  # 4096 flattened rows
RS = R // NCORES           # 512 output rows per core
KS = C // 128              # 8 contraction blocks of 128
CHUNK = 512                # rows per x^T/qkv chunk
NCH = R // CHUNK           # 8 chunks
NQT = N // 128             # 16 query tiles per batch
NKT = N // 128             # 16 key tiles per batch
F32 = mybir.dt.float32
F16 = mybir.dt.float16

INV_SCALE = float(np.sqrt(D))  # 8.0, multiplied into q


def _bcast(ap, parts):
    """Broadcast a DRAM AP across `parts` partitions (step-0 partition dim)."""
    return bass.AP(tensor=ap.tensor, offset=ap.offset,
                   ap=[[0, parts]] + list(ap.ap))


def build_program():
    nc = bacc.Bacc("TRN2", target_bir_lowering=False, debug=False,
                   num_devices=NCORES)

    xT_in = nc.dram_tensor("xT", [C, R], F32, kind="ExternalInput")
    wq_in = nc.dram_tensor("wq", [C, D2], F32, kind="ExternalInput")
    wk_in = nc.dram_tensor("wk", [C, D2], F32, kind="ExternalInput")
    wv_in = nc.dram_tensor("wv", [C, D2], F32, kind="ExternalInput")
    bq_in = nc.dram_tensor("bq", [D2], F32, kind="ExternalInput")
    bk_in = nc.dram_tensor("bk", [D2], F32, kind="ExternalInput")
    bv_in = nc.dram_tensor("bv", [D2], F32, kind="ExternalInput")
    wp_in = nc.dram_tensor("wp", [C, C], F32, kind="ExternalInput")
    bp_in = nc.dram_tensor("bp", [C], F32, kind="ExternalInput")
    out_t = nc.dram_tensor("out", [RS, C], F32, kind="ExternalOutput")

    with tile.TileContext(nc) as tc:
        kernel_body(tc, xT_in, wq_in, wk_in, wv_in, bq_in, bk_in, bv_in,
                    wp_in, bp_in, out_t)
    nc.compile()
    return nc


def kernel_body(tc, xT_in, wq_in, wk_in, wv_in, bq_in, bk_in, bv_in,
                wp_in, bp_in, out_t):
    nc = tc.nc
    Exp = mybir.ActivationFunctionType.Exp
    Ident = mybir.ActivationFunctionType.Identity

    ctx = ExitStack()
    consts = ctx.enter_context(tc.tile_pool(name="consts", bufs=1))
    persist = ctx.enter_context(tc.tile_pool(name="persist", bufs=1))
    dram = ctx.enter_context(tc.tile_pool(name="dram", bufs=1, space="DRAM"))

    ident = consts.tile([128, 128], F32)
    make_identity(nc, ident)
    ones64 = consts.tile([1, 64], F32)
    nc.vector.memset(ones64, 1.0)

    # --- weights / biases for qkv (hi/lo split in fp16) ---
    with tc.tile_pool(name="wstage", bufs=2) as wstage:
        def split_w(name, t_in):
            w_f32 = wstage.tile([128, KS, D2], F32, tag="w_f32", name=name)
            nc.sync.dma_start(w_f32,
                              t_in.ap().rearrange("(ks p) m -> p ks m", p=128))
            hi = consts.tile([128, KS, D2], F16, name=name + "_hi")
            lo = consts.tile([128, KS, D2], F16, name=name + "_lo")
            nc.vector.tensor_copy(hi, w_f32)
            nc.vector.tensor_sub(lo, w_f32, hi)
            return hi, lo

        wq_hi, wq_lo = split_w("wq", wq_in)
        wk_hi, wk_lo = split_w("wk", wk_in)
        wv_f32 = wstage.tile([128, KS, D2], F32, tag="w_f32", name="wv")
        nc.sync.dma_start(wv_f32, wv_in.ap().rearrange("(ks p) m -> p ks m", p=128))
        wv_bf = consts.tile([128, KS, D2], F16)
        nc.vector.tensor_copy(wv_bf, wv_f32)

    bq_sb = consts.tile([128, 1], F32)
    bk_sb = consts.tile([128, 1], F32)
    nc.sync.dma_start(bq_sb, bq_in.ap().rearrange("(p o) -> p o", o=1))
    nc.sync.dma_start(bk_sb, bk_in.ap().rearrange("(p o) -> p o", o=1))
    bq8_sb = consts.tile([128, 1], F32)
    nc.scalar.mul(bq8_sb, bq_sb, INV_SCALE)
    bv_sb = consts.tile([128, D2], F32)
    nc.sync.dma_start(bv_sb, _bcast(bv_in.ap(), 128))

    # --- persistent activations (fp16) ---
    qT_hi = persist.tile([128, R], F16)   # sqrt(d) * q^T, hi part
    qT_lo = persist.tile([128, R], F16)
    kT_hi = persist.tile([128, R], F16)
    kT_lo = persist.tile([128, R], F16)
    # v with a ones column per head: [p, row_tile, head, 65]
    v_sb = persist.tile([128, R // 128, HPC, D + 1], F16)
    nc.vector.memset(v_sb[:, :, :, D:D + 1], 1.0)
    outT_sb = persist.tile([128, R], F16)

    SEQ = [(b, h) for b in range(B) for h in range(HPC)]
    attctx = ExitStack()
    attE = ctx.enter_context(tc.tile_pool(name="attE", bufs=1))

    stats_t = {}   # stage -> stats tile [128, 16] (negated row maxes)
    biasT_t = {}   # stage -> [1, N] flattened -rowmax
    kext_t = {}    # stage -> (kThx, kTlx)
    av_t = {}      # stage -> [65, 4, 512] unnormalized head outputs + sums

    def emit_nat_mt(i, mt, pool, ps_bufs, tag="ps"):
        b, h = SEQ[i]
        c0 = b * N
        hp = h * D
        if mt == 0:
            stats_t[i] = attE.tile([128, 16], F32, tag="stats", bufs=2,
                                  name=f"stats{i}")
        stats = stats_t[i]
        q_l = qT_hi[hp:hp + D, c0 + mt * 128: c0 + (mt + 1) * 128]
        mx = attE.tile([128, 4], F32, tag="mx" + tag, bufs=2, name="mx")
        for j in range(4):
            p = pool.tile([128, 512], F32, tag=tag, bufs=ps_bufs, name="ps")
            nc.tensor.matmul(
                p, q_l,
                kT_hi[hp:hp + D, c0 + j * 512:c0 + (j + 1) * 512],
                start=True, stop=True)
            nc.vector.reduce_max(mx[:, j:j + 1], p,
                                 axis=mybir.AxisListType.X)
        nc.vector.reduce_max(stats[:, mt:mt + 1], mx,
                             axis=mybir.AxisListType.X, negate=True)

    def emit_nat_pair(ia, ib, mt):
        # two heads on disjoint PE row groups -> the 8 matmuls run pairwise
        # concurrently in the array
        ba, ha = SEQ[ia]
        bb, hb = SEQ[ib]
        assert ha != hb
        for i, tag in ((ia, "psA"), (ib, "psB")):
            if mt == 0:
                stats_t[i] = attE.tile([128, 16], F32, tag="stats", bufs=2,
                                       name=f"stats{i}")
        mxs = {}
        for i, tag in ((ia, "psA"), (ib, "psB")):
            mxs[i] = attE.tile([128, 4], F32, tag="mx" + tag, bufs=2, name="mx")
        ps = {}
        for j in range(4):
            for i, tag in ((ia, "psA"), (ib, "psB")):
                b, h = SEQ[i]
                c0 = b * N
                hp = h * D
                q_l = qT_hi[hp:hp + D, c0 + mt * 128: c0 + (mt + 1) * 128]
                p = p3.tile([128, 512], F32, tag=tag, bufs=2, name="ps")
                nc.tensor.matmul(
                    p, q_l,
                    kT_hi[hp:hp + D, c0 + j * 512:c0 + (j + 1) * 512],
                    start=True, stop=True)
                ps[i] = p
            for i, tag in ((ia, "psA"), (ib, "psB")):
                nc.vector.reduce_max(mxs[i][:, j:j + 1], ps[i],
                                     axis=mybir.AxisListType.X)
        for i, tag in ((ia, "psA"), (ib, "psB")):
            nc.vector.reduce_max(stats_t[i][:, mt:mt + 1], mxs[i],
                                 axis=mybir.AxisListType.X, negate=True)

    def emit_stats_flatten(i, pool, tag):
        stats = stats_t.pop(i)
        pstat = pool.tile([16, 128], F32, tag=tag, bufs=1, name="pstat")
        nc.tensor.transpose(pstat, stats, ident)
        statsT = attE.tile([16, 128], F16, tag="statsT", bufs=2, name="statsT")
        nc.vector.tensor_copy(statsT, pstat)
        biasT = attE.tile([1, N], F16, tag="biasT", bufs=2, name="biasT")
        nc.sync.dma_start(biasT.rearrange("s (m q) -> s m q", m=16), statsT)
        biasT_t[i] = biasT

    # ---------- Phase 1+2: x^T chunks and QKV projections ----------
    xT_view = xT_in.ap().rearrange("(ks p) r -> p ks r", p=128)
    with tc.tile_pool(name="xload", bufs=2) as xload, \
         tc.tile_pool(name="p12", bufs=1, space="PSUM") as p12:
        for ch in range(NCH):
            r0 = ch * CHUNK
            xT = xload.tile([128, KS, CHUNK], F32, tag="xT")
            xT_hi = xload.tile([128, KS, CHUNK], F16, tag="xT_hi")
            xT_lo = xload.tile([128, KS, CHUNK], F16, tag="xT_lo")
            # fp16 hi/lo split (hi cast on ACT, lo residual on DVE),
            # in halves so compute starts before the full chunk lands
            for hf in range(2):
                ksl = slice(hf * KS // 2, (hf + 1) * KS // 2)
                nc.sync.dma_start(xT[:, ksl], xT_view[:, ksl, r0:r0 + CHUNK])
                nc.scalar.copy(xT_hi[:, ksl], xT[:, ksl])
                nc.vector.tensor_sub(xT_lo[:, ksl], xT[:, ksl], xT_hi[:, ksl])
            # q^T, k^T for this chunk (128 rows = 2 heads * 64), 3-pass split
            for (whi, wlo, dst_hi, dst_lo, bias, scale) in (
                    (wq_hi, wq_lo, qT_hi, qT_lo, bq8_sb, INV_SCALE),
                    (wk_hi, wk_lo, kT_hi, kT_lo, bk_sb, 1.0)):
                pqk = p12.tile([128, CHUNK], F32, tag="pqk", bufs=2)
                passes = [(whi, xT_hi), (whi, xT_lo), (wlo, xT_hi)]
                for pi, (w_p, x_p) in enumerate(passes):
                    for ks in range(KS):
                        nc.tensor.matmul(pqk, w_p[:, ks], x_p[:, ks],
                                         start=(pi == 0 and ks == 0),
                                         stop=(pi == 2 and ks == KS - 1))
                # hi = f16(scale*psum + bias); lo = f16((scale*psum+bias) - hi)
                nc.scalar.activation(dst_hi[:, r0:r0 + CHUNK], pqk, Ident,
                                     bias=bias, scale=scale)
                tmp = xload.tile([128, CHUNK], F32, tag="qk_tmp", bufs=2)
                nc.vector.tensor_scalar(tmp, pqk, scalar1=scale, scalar2=bias,
                                        op0=mybir.AluOpType.mult,
                                        op1=mybir.AluOpType.add)
                nc.gpsimd.tensor_sub(dst_lo[:, r0:r0 + CHUNK], tmp,
                                     dst_hi[:, r0:r0 + CHUNK])
            # v natural layout (single-pass fp16)
            for m in range(4):
                pv = p12.tile([128, D2], F32, tag="pv", bufs=1)
                for ks in range(KS):
                    nc.tensor.matmul(pv, xT_hi[:, ks, m * 128:(m + 1) * 128],
                                     wv_bf[:, ks],
                                     start=(ks == 0), stop=(ks == KS - 1))
                for h in range(HPC):
                    nc.vector.tensor_add(
                        v_sb[:, ch * 4 + m, h, 0:D],
                        pv[:, h * D:(h + 1) * D],
                        bv_sb[:, h * D:(h + 1) * D])
                del pv
                # batch-0 q/k ready after chunk 3: overlap stage-0 max pass
                if ch >= NCH // 2:
                    emit_nat_mt(0, (ch - NCH // 2) * 4 + m, p12, 2)

    # projection weights: load + cast early so the post-A2A path is short
    projp = ctx.enter_context(tc.tile_pool(name="projp", bufs=1))
    wp_bf = projp.tile([128, KS, C], F16)
    with tc.tile_pool(name="wpl", bufs=2) as wpl:
        for ks in range(KS):
            wp_chunk = wpl.tile([128, C], F32, tag="wp_chunk")
            nc.sync.dma_start(
                wp_chunk, wp_in.ap()[ks * 128:(ks + 1) * 128, :])
            nc.vector.tensor_copy(wp_bf[:, ks], wp_chunk)
    bp_sb = projp.tile([128, C], F32)
    nc.sync.dma_start(bp_sb, _bcast(bp_in.ap(), 128))

    # ---------- Phase 3: attention, software-pipelined over 4 stages ----------

    def emit_kext(i):
        b, h = SEQ[i]
        c0 = b * N
        hp = h * D
        kThh = att.tile([128, N], F16, tag="kThh", bufs=2, name="kThh")
        kTcross = att.tile([128, N], F16, tag="kTcross", bufs=2, name="kTcross")
        # k_hi twice (disjoint row-groups -> adjacent hi*hi matmuls of a kt
        # pair run concurrently in the PE array)
        nc.vector.tensor_copy(kThh[0:64, :], kT_hi[hp:hp + D, c0:c0 + N])
        nc.vector.tensor_copy(kThh[64:128, :], kT_hi[hp:hp + D, c0:c0 + N])
        # cross operand: k_hi (64) | k_lo (63) | ones (bias row); one lo*hi
        # dim is sacrificed for the bias row (~2^-11 relative score effect).
        # Engine APs need 32-aligned partition bases, so the single ones row
        # at partition 127 is laid down by a memset over [96:128) that the
        # k_lo copy then overwrites up to row 126.
        nc.vector.tensor_copy(kTcross[0:64, :], kT_hi[hp:hp + D, c0:c0 + N])
        nc.vector.memset(kTcross[96:128, :], 1.0)
        nc.vector.tensor_copy(kTcross[64:127, :],
                              kT_lo[hp:hp + D - 1, c0:c0 + N])
        kext_t[i] = (kThh, kTcross)

    def emit_T_j(i, j, fillers):
        b, h = SEQ[i]
        c0 = b * N
        hp = h * D
        kThh, kTcross = kext_t[i]
        biasT = biasT_t[i]
        if j == 0:
            av_t[i] = att.tile([65, 4, 512], F32, tag="av", bufs=2,
                               name=f"av{i}")
        qs = slice(c0 + j * 512, c0 + (j + 1) * 512)
        qhx = att.tile([128, 512], F16, tag="qhx", bufs=2, name="qhx")
        qcross = att.tile([128, 512], F16, tag="qcross", bufs=2, name="qcross")
        nc.vector.tensor_copy(qhx[0:64, :], qT_hi[hp:hp + D, qs])
        nc.vector.tensor_copy(qhx[64:128, :], qT_hi[hp:hp + D, qs])
        # cross rhs: q_lo (64) | q_hi (63) | bias. The bias row lands on the
        # unaligned partition 127 via DMA (engine APs need 32-aligned bases)
        nc.vector.tensor_copy(qcross[0:64, :], qT_lo[hp:hp + D, qs])
        nc.vector.tensor_copy(qcross[64:127, :], qT_hi[hp:hp + D - 1, qs])
        nc.sync.dma_start(qcross[127:128, :],
                          biasT[0:1, j * 512:(j + 1) * 512])
        eT = eTp.tile([128, NKT, 512], F16, tag="eT", name="eT")
        for ktp in range(NKT // 2):
            kt0, kt1 = 2 * ktp, 2 * ktp + 1
            psT0 = p3.tile([128, 512], F32, tag="psT", bufs=3, name="psT")
            psT1 = p3.tile([128, 512], F32, tag="psT", bufs=3, name="psT")
            sl0 = slice(kt0 * 128, (kt0 + 1) * 128)
            sl1 = slice(kt1 * 128, (kt1 + 1) * 128)
            # the two K=64 hi*hi matmuls sit on disjoint row-groups and run
            # concurrently; the K=128 cross matmul carries hi*lo + lo*hi +
            # the -rowmax bias row
            nc.tensor.matmul(psT0, kThh[0:64, sl0], qhx[0:64, :],
                             start=True, stop=False)
            nc.tensor.matmul(psT1, kThh[64:128, sl1], qhx[64:128, :],
                             start=True, stop=False)
            nc.tensor.matmul(psT0, kTcross[:, sl0], qcross,
                             start=False, stop=True)
            nc.tensor.matmul(psT1, kTcross[:, sl1], qcross,
                             start=False, stop=True)
            nc.scalar.activation(eT[:, kt0], psT0, Exp)
            nc.scalar.activation(eT[:, kt1], psT1, Exp)
            # sprinkle next-stage max-pass work between score groups so the
            # PE never stalls in a burst behind the DVE reduce queue
            if ktp % 2 == 1 and fillers:
                fillers.pop(0)()
        pav = p3.tile([65, 512], F32, tag="pav", bufs=1, name="pav")
        for kt in range(NKT):
            nc.tensor.matmul(pav, v_sb[:, b * 16 + kt, h, :], eT[:, kt],
                             start=(kt == 0), stop=(kt == NKT - 1))
        nc.vector.tensor_copy(av_t[i][:, j], pav)

    def emit_norm(i):
        b, h = SEQ[i]
        c0 = b * N
        hp = h * D
        av = av_t.pop(i)
        # 1/s = exp(-ln s) on ACT (a DVE reciprocal on a single-partition row
        # costs ~3.3us), then broadcast across 64 partitions via a K=1 outer
        # product on the PE (which is idle at stage boundaries) so the gpsimd
        # queue stays empty for the collective doorbell
        rj0 = att.tile([1, 4, 512], F32, tag="rj0", bufs=2, name="rj0")
        nc.vector.tensor_copy(rj0, av[64:65, :, :])
        nc.scalar.activation(rj0, rj0, mybir.ActivationFunctionType.Ln)
        nc.scalar.activation(rj0, rj0, mybir.ActivationFunctionType.Exp,
                             scale=-1.0)
        for j in range(4):
            qs = slice(c0 + j * 512, c0 + (j + 1) * 512)
            rb = p3.tile([64, 512], F32, tag="pav", bufs=1, name="rb")
            nc.tensor.matmul(rb, ones64[0:1, :], rj0[0:1, j],
                             start=True, stop=True)
            nc.vector.tensor_mul(outT_sb[hp:hp + D, qs], av[0:64, j], rb)

    # pipeline: stage i's transposed pass interleaves stage i+1's max pass
    att = attctx.enter_context(tc.tile_pool(name="att", bufs=1))
    eTp = attctx.enter_context(tc.tile_pool(name="eTp", bufs=2))
    p3 = attctx.enter_context(tc.tile_pool(name="p3", bufs=1, space="PSUM"))
    emit_stats_flatten(0, p3, "pav")
    emit_kext(0)
    # per-batch AllToAll: batch b's shard j = its rows [j*256,(j+1)*256)
    # (core j's output = batch0 rows j*256.. plus batch1 rows j*256..)
    HRS = RS // 2  # 256 rows per batch per core
    a2a_in = [dram.tile([NCORES * 128, HRS], F16, name=f"a2ai{b}")
              for b in range(B)]
    a2a_out = [dram.tile([NCORES * 128, HRS], F16, name=f"a2ao{b}")
               for b in range(B)]
    lhsT_proj = [projp.tile([128, KS, HRS], F16, name=f"lhsTp{b}")
                 for b in range(B)]

    def emit_a2a(b):
        nc.sync.dma_start(
            a2a_in[b].rearrange("(j p) r -> p j r", j=NCORES),
            outT_sb[:, b * N:(b + 1) * N].rearrange("p (j r) -> p j r",
                                                    j=NCORES))
        nc.gpsimd.collective_compute(
            "AllToAll", mybir.AluOpType.bypass,
            replica_groups=[list(range(NCORES))],
            ins=[a2a_in[b][:]], outs=[a2a_out[b][:]])
        nc.sync.dma_start(
            lhsT_proj[b],
            a2a_out[b].rearrange("(j p) r -> p j r", j=NCORES))

    fill_by_stage = {
        0: [(lambda mt=mt: emit_nat_pair(1, 2, mt)) for mt in range(NQT)],
        1: [(lambda mt=mt: emit_nat_mt(3, mt, p3, 2, tag="psA"))
            for mt in range(NQT // 2)],
        2: [(lambda mt=mt: emit_nat_mt(3, mt, p3, 2, tag="psA"))
            for mt in range(NQT // 2, NQT)],
        3: [],
    }
    for i in range(4):
        fillers = fill_by_stage[i]
        for j in range(4):
            emit_T_j(i, j, fillers)
        while fillers:
            fillers.pop(0)()
        if i == 0:
            emit_stats_flatten(1, p3, "pav")
            emit_kext(1)
            emit_stats_flatten(2, p3, "pav")
        elif i == 1:
            emit_kext(2)
        elif i == 2:
            emit_stats_flatten(3, p3, "pav")
            emit_kext(3)
        emit_norm(i)
        if i == 1:
            emit_a2a(0)
        elif i == 3:
            emit_a2a(1)
    attctx.close()

    # ---------- Phase 4: output projection (batch-0 rows overlap A2A #2) ----
    with tc.tile_pool(name="proj", bufs=1) as proj, \
         tc.tile_pool(name="p4", bufs=1, space="PSUM") as p4:
        for m in range(RS // 128):
            lhsT_b = lhsT_proj[m // 2]
            mo = (m % 2) * 128
            for nt in range(C // 512):
                pp = p4.tile([128, 512], F32, tag="pp", bufs=4)
                for ks in range(KS):
                    nc.tensor.matmul(pp, lhsT_b[:, ks, mo:mo + 128],
                                     wp_bf[:, ks, nt * 512:(nt + 1) * 512],
                                     start=(ks == 0), stop=(ks == KS - 1))
                o_sb = proj.tile([128, 512], F32, tag="o_sb", bufs=4)
                nc.vector.tensor_add(o_sb, pp,
                                     bp_sb[:, nt * 512:(nt + 1) * 512])
                nc.sync.dma_start(
                    out_t.ap()[m * 128:(m + 1) * 128, nt * 512:(nt + 1) * 512],
                    o_sb)
    ctx.close()


_PROGRAM = None


def _get_program():
    global _PROGRAM
    if _PROGRAM is None:
        _PROGRAM = build_program()
    return _PROGRAM


def kernel(x, W_qkv, b_qkv, W_proj, b_proj, _trace=False):
    xT = np.ascontiguousarray(np.asarray(x, dtype=np.float32).reshape(R, C).T)
    W_qkv = np.asarray(W_qkv, dtype=np.float32)
    b_qkv = np.asarray(b_qkv, dtype=np.float32)
    W_proj = np.ascontiguousarray(np.asarray(W_proj, dtype=np.float32))
    b_proj = np.ascontiguousarray(np.asarray(b_proj, dtype=np.float32))

    in_maps = []
    for i in range(NCORES):
        lo = i * D2            # first column of my heads within a qkv block
        hi = lo + D2
        in_maps.append({
            "xT": xT,
            "wq": np.ascontiguousarray(W_qkv[:, 0 * C + lo:0 * C + hi]),
            "wk": np.ascontiguousarray(W_qkv[:, 1 * C + lo:1 * C + hi]),
            "wv": np.ascontiguousarray(W_qkv[:, 2 * C + lo:2 * C + hi]),
            "bq": np.ascontiguousarray(b_qkv[0 * C + lo:0 * C + hi]),
            "bk": np.ascontiguousarray(b_qkv[1 * C + lo:1 * C + hi]),
            "bv": np.ascontiguousarray(b_qkv[2 * C + lo:2 * C + hi]),
            "wp": W_proj,
            "bp": b_proj,
        })

    nc = _get_program()
    res = bass_utils.run_bass_kernel_spmd(
        nc, in_maps, core_ids=list(range(NCORES)), trace=_trace)
    out = np.empty((R, C), dtype=np.float32)
    HRS = RS // 2
    for i in range(NCORES):
        o = res.results[i]["out"]
        for b in range(B):
            out[b * N + i * HRS: b * N + (i + 1) * HRS] = \
                o[b * HRS:(b + 1) * HRS]
    if _trace:
        kernel.last_results = res
    return out.reshape(B, N, C)


# revision 27
# speedup vs baseline: 1.0251x; 1.0251x over previous
"""Trainium2 Bass kernel for 16-head MHA (b=2, n=2048, c=1024, d=64).

Reference semantics (note the inverted scale "bug" reproduced faithfully):
    qkv = x @ W_qkv + b_qkv
    scores = (q @ k^T) * sqrt(d)          # multiplied, not divided
    out = softmax(scores) @ v
    y = concat_heads(out) @ W_proj + b_proj

Sharding: tensor-parallel over heads. Each of the 8 cores computes QKV +
attention for its 2 heads (head-dim-transposed layouts so no activation
transposes are needed beyond one x^T pass), then a single AllToAll moves the
per-head attention outputs into a row-sharded layout and each core computes
the final projection for its 512-row output shard. Host concatenates shards.

Precision strategy: exactness is needed only upstream of exp (the *sqrt(d)
score scale amplifies absolute errors into exp-space relative errors).
Those matmuls use an fp16 hi/lo split, 3 accumulating passes
(hi*hi + hi*lo + lo*hi; the dropped lo*lo term is negligible) — products
are exact in the PE and accumulate in fp32. The row-max pass needs no
precision at all (softmax renormalization cancels any max offset exactly),
so it runs single-pass on the hi parts. exp scores, A@V and the projection
are plain fp16 (inputs exact to ~1e-3, no amplification).

Softmax plumbing: the transposed-score matmul gets an extra contraction row
(ones in k^T_ext, -rowmax in q_ext) so exp(psum) needs no separate bias op;
V gets an extra ones *column* so the A@V matmul also yields the softmax
denominators; one reciprocal + gpsimd partition-broadcast + multiply
normalizes each head-output block (emitted late so the DVE reciprocal never
blocks the PE-feeding copy chain).

The attention phase is software-pipelined at emission level: the row-max
pass of stage i+1 is interleaved with the transposed pass of stage i, so
the PE never drains while the stats chain (transpose/flatten/DMA) of the
next stage resolves.
"""

import sys
from contextlib import ExitStack

sys.path.insert(0, "/opt/trn_rl_repo")

import numpy as np

import concourse.bass as bass
import concourse.tile as tile
from concourse import bacc, mybir
from concourse import bass_utils
from concourse.masks import make_identity

# Problem shape (hardcoded per contract)
B, N, C = 2, 2048, 1024
H, D = 16, 64
NCORES = 8
HPC = H // NCORES          # heads per core = 2
D2 = HPC * D               # 128 = per-core slice of the concat dim
R = B * N                  # 4096 flattened rows
RS = R // NCORES           # 512 output rows per core
KS = C // 128              # 8 contraction blocks of 128
CHUNK = 512                # rows per x^T/qkv chunk
NCH = R // CHUNK           # 8 chunks
NQT = N // 128             # 16 query tiles per batch
NKT = N // 128             # 16 key tiles per batch
F32 = mybir.dt.float32
F16 = mybir.dt.float16

INV_SCALE = float(np.sqrt(D))  # 8.0, multiplied into q


def _bcast(ap, parts):
    """Broadcast a DRAM AP across `parts` partitions (step-0 partition dim)."""
    return bass.AP(tensor=ap.tensor, offset=ap.offset,
                   ap=[[0, parts]] + list(ap.ap))


def build_program():
    nc = bacc.Bacc("TRN2", target_bir_lowering=False, debug=False,
                   num_devices=NCORES)

    xT_in = nc.dram_tensor("xT", [C, R], F32, kind="ExternalInput")
    wq_in = nc.dram_tensor("wq", [C, D2], F32, kind="ExternalInput")
    wk_in = nc.dram_tensor("wk", [C, D2], F32, kind="ExternalInput")
    wv_in = nc.dram_tensor("wv", [C, D2], F32, kind="ExternalInput")
    bq_in = nc.dram_tensor("bq", [D2], F32, kind="ExternalInput")
    bk_in = nc.dram_tensor("bk", [D2], F32, kind="ExternalInput")
    bv_in = nc.dram_tensor("bv", [D2], F32, kind="ExternalInput")
    wp_in = nc.dram_tensor("wp", [C, C], F32, kind="ExternalInput")
    bp_in = nc.dram_tensor("bp", [C], F32, kind="ExternalInput")
    out_t = nc.dram_tensor("out", [RS, C], F32, kind="ExternalOutput")

    with tile.TileContext(nc) as tc:
        kernel_body(tc, xT_in, wq_in, wk_in, wv_in, bq_in, bk_in, bv_in,
                    wp_in, bp_in, out_t)
    nc.compile()
    return nc


def kernel_body(tc, xT_in, wq_in, wk_in, wv_in, bq_in, bk_in, bv_in,
                wp_in, bp_in, out_t):
    nc = tc.nc
    Exp = mybir.ActivationFunctionType.Exp
    Ident = mybir.ActivationFunctionType.Identity

    ctx = ExitStack()
    consts = ctx.enter_context(tc.tile_pool(name="consts", bufs=1))
    persist = ctx.enter_context(tc.tile_pool(name="persist", bufs=1))
    dram = ctx.enter_context(tc.tile_pool(name="dram", bufs=1, space="DRAM"))

    ident = consts.tile([128, 128], F32)
    make_identity(nc, ident)
    ones64 = consts.tile([1, 64], F32)
    nc.vector.memset(ones64, 1.0)

    # --- weights / biases for qkv (hi/lo split in fp16) ---
    with tc.tile_pool(name="wstage", bufs=2) as wstage:
        def split_w(name, t_in):
            w_f32 = wstage.tile([128, KS, D2], F32, tag="w_f32", name=name)
            nc.sync.dma_start(w_f32,
                              t_in.ap().rearrange("(ks p) m -> p ks m", p=128))
            hi = consts.tile([128, KS, D2], F16, name=name + "_hi")
            lo = consts.tile([128, KS, D2], F16, name=name + "_lo")
            nc.vector.tensor_copy(hi, w_f32)
            nc.vector.tensor_sub(lo, w_f32, hi)
            return hi, lo

        wq_hi, wq_lo = split_w("wq", wq_in)
        wk_hi, wk_lo = split_w("wk", wk_in)
        wv_f32 = wstage.tile([128, KS, D2], F32, tag="w_f32", name="wv")
        nc.sync.dma_start(wv_f32, wv_in.ap().rearrange("(ks p) m -> p ks m", p=128))
        wv_bf = consts.tile([128, KS, D2], F16)
        nc.vector.tensor_copy(wv_bf, wv_f32)

    bq_sb = consts.tile([128, 1], F32)
    bk_sb = consts.tile([128, 1], F32)
    nc.sync.dma_start(bq_sb, bq_in.ap().rearrange("(p o) -> p o", o=1))
    nc.sync.dma_start(bk_sb, bk_in.ap().rearrange("(p o) -> p o", o=1))
    bq8_sb = consts.tile([128, 1], F32)
    nc.scalar.mul(bq8_sb, bq_sb, INV_SCALE)
    bv_sb = consts.tile([128, D2], F32)
    nc.sync.dma_start(bv_sb, _bcast(bv_in.ap(), 128))

    # --- persistent activations (fp16) ---
    qT_hi = persist.tile([128, R], F16)   # sqrt(d) * q^T, hi part
    qT_lo = persist.tile([128, R], F16)
    kT_hi = persist.tile([128, R], F16)
    kT_lo = persist.tile([128, R], F16)
    # v with a ones column per head: [p, row_tile, head, 65]
    v_sb = persist.tile([128, R // 128, HPC, D + 1], F16)
    nc.vector.memset(v_sb[:, :, :, D:D + 1], 1.0)
    outT_sb = persist.tile([128, R], F16)

    SEQ = [(b, h) for b in range(B) for h in range(HPC)]
    attctx = ExitStack()
    attE = ctx.enter_context(tc.tile_pool(name="attE", bufs=1))

    stats_t = {}   # stage -> stats tile [128, 16] (negated row maxes)
    biasT_t = {}   # stage -> [1, N] flattened -rowmax
    kext_t = {}    # stage -> (kThx, kTlx)
    av_t = {}      # stage -> [65, 4, 512] unnormalized head outputs + sums

    def emit_nat_mt(i, mt, pool, ps_bufs, tag="ps"):
        b, h = SEQ[i]
        c0 = b * N
        hp = h * D
        if mt == 0:
            stats_t[i] = attE.tile([128, 16], F32, tag="stats", bufs=2,
                                  name=f"stats{i}")
        stats = stats_t[i]
        q_l = qT_hi[hp:hp + D, c0 + mt * 128: c0 + (mt + 1) * 128]
        mx = attE.tile([128, 4], F32, tag="mx" + tag, bufs=2, name="mx")
        for j in range(4):
            p = pool.tile([128, 512], F32, tag=tag, bufs=ps_bufs, name="ps")
            nc.tensor.matmul(
                p, q_l,
                kT_hi[hp:hp + D, c0 + j * 512:c0 + (j + 1) * 512],
                start=True, stop=True)
            nc.vector.reduce_max(mx[:, j:j + 1], p,
                                 axis=mybir.AxisListType.X)
        nc.vector.reduce_max(stats[:, mt:mt + 1], mx,
                             axis=mybir.AxisListType.X, negate=True)

    def emit_nat_pair(ia, ib, mt):
        # two heads on disjoint PE row groups -> the 8 matmuls run pairwise
        # concurrently in the array
        ba, ha = SEQ[ia]
        bb, hb = SEQ[ib]
        assert ha != hb
        for i, tag in ((ia, "psA"), (ib, "psB")):
            if mt == 0:
                stats_t[i] = attE.tile([128, 16], F32, tag="stats", bufs=2,
                                       name=f"stats{i}")
        mxs = {}
        for i, tag in ((ia, "psA"), (ib, "psB")):
            mxs[i] = attE.tile([128, 4], F32, tag="mx" + tag, bufs=2, name="mx")
        ps = {}
        for j in range(4):
            for i, tag in ((ia, "psA"), (ib, "psB")):
                b, h = SEQ[i]
                c0 = b * N
                hp = h * D
                q_l = qT_hi[hp:hp + D, c0 + mt * 128: c0 + (mt + 1) * 128]
                p = p3.tile([128, 512], F32, tag=tag, bufs=2, name="ps")
                nc.tensor.matmul(
                    p, q_l,
                    kT_hi[hp:hp + D, c0 + j * 512:c0 + (j + 1) * 512],
                    start=True, stop=True)
                ps[i] = p
            for i, tag in ((ia, "psA"), (ib, "psB")):
                nc.vector.reduce_max(mxs[i][:, j:j + 1], ps[i],
                                     axis=mybir.AxisListType.X)
        for i, tag in ((ia, "psA"), (ib, "psB")):
            nc.vector.reduce_max(stats_t[i][:, mt:mt + 1], mxs[i],
                                 axis=mybir.AxisListType.X, negate=True)

    def emit_stats_flatten(i, pool, tag):
        stats = stats_t.pop(i)
        pstat = pool.tile([16, 128], F32, tag=tag, bufs=1, name="pstat")
        nc.tensor.transpose(pstat, stats, ident)
        statsT = attE.tile([16, 128], F16, tag="statsT", bufs=2, name="statsT")
        nc.vector.tensor_copy(statsT, pstat)
        biasT = attE.tile([1, N], F16, tag="biasT", bufs=2, name="biasT")
        nc.sync.dma_start(biasT.rearrange("s (m q) -> s m q", m=16), statsT)
        biasT_t[i] = biasT

    # ---------- Phase 1+2: x^T chunks and QKV projections ----------
    xT_view = xT_in.ap().rearrange("(ks p) r -> p ks r", p=128)
    with tc.tile_pool(name="xload", bufs=2) as xload, \
         tc.tile_pool(name="p12", bufs=1, space="PSUM") as p12:
        for ch in range(NCH):
            r0 = ch * CHUNK
            xT = xload.tile([128, KS, CHUNK], F32, tag="xT")
            xT_hi = xload.tile([128, KS, CHUNK], F16, tag="xT_hi")
            xT_lo = xload.tile([128, KS, CHUNK], F16, tag="xT_lo")
            # fp16 hi/lo split (hi cast on ACT, lo residual on DVE),
            # in halves so compute starts before the full chunk lands
            for hf in range(2):
                ksl = slice(hf * KS // 2, (hf + 1) * KS // 2)
                nc.sync.dma_start(xT[:, ksl], xT_view[:, ksl, r0:r0 + CHUNK])
                nc.scalar.copy(xT_hi[:, ksl], xT[:, ksl])
                nc.vector.tensor_sub(xT_lo[:, ksl], xT[:, ksl], xT_hi[:, ksl])
            # q^T, k^T for this chunk (128 rows = 2 heads * 64), 3-pass split
            for (whi, wlo, dst_hi, dst_lo, bias, scale) in (
                    (wq_hi, wq_lo, qT_hi, qT_lo, bq8_sb, INV_SCALE),
                    (wk_hi, wk_lo, kT_hi, kT_lo, bk_sb, 1.0)):
                pqk = p12.tile([128, CHUNK], F32, tag="pqk", bufs=2)
                passes = [(whi, xT_hi), (whi, xT_lo), (wlo, xT_hi)]
                for pi, (w_p, x_p) in enumerate(passes):
                    for ks in range(KS):
                        nc.tensor.matmul(pqk, w_p[:, ks], x_p[:, ks],
                                         start=(pi == 0 and ks == 0),
                                         stop=(pi == 2 and ks == KS - 1))
                # hi = f16(scale*psum + bias); lo = f16((scale*psum+bias) - hi)
                nc.scalar.activation(dst_hi[:, r0:r0 + CHUNK], pqk, Ident,
                                     bias=bias, scale=scale)
                tmp = xload.tile([128, CHUNK], F32, tag="qk_tmp", bufs=2)
                nc.vector.tensor_scalar(tmp, pqk, scalar1=scale, scalar2=bias,
                                        op0=mybir.AluOpType.mult,
                                        op1=mybir.AluOpType.add)
                nc.gpsimd.tensor_sub(dst_lo[:, r0:r0 + CHUNK], tmp,
                                     dst_hi[:, r0:r0 + CHUNK])
            # v natural layout (single-pass fp16)
            for m in range(4):
                pv = p12.tile([128, D2], F32, tag="pv", bufs=1)
                for ks in range(KS):
                    nc.tensor.matmul(pv, xT_hi[:, ks, m * 128:(m + 1) * 128],
                                     wv_bf[:, ks],
                                     start=(ks == 0), stop=(ks == KS - 1))
                for h in range(HPC):
                    nc.vector.tensor_add(
                        v_sb[:, ch * 4 + m, h, 0:D],
                        pv[:, h * D:(h + 1) * D],
                        bv_sb[:, h * D:(h + 1) * D])
                del pv
                # batch-0 q/k ready after chunk 3: overlap stage-0 max pass
                if ch >= NCH // 2:
                    emit_nat_mt(0, (ch - NCH // 2) * 4 + m, p12, 2)

    # projection weights: load + cast early so the post-A2A path is short
    projp = ctx.enter_context(tc.tile_pool(name="projp", bufs=1))
    wp_bf = projp.tile([128, KS, C], F16)
    with tc.tile_pool(name="wpl", bufs=2) as wpl:
        for ks in range(KS):
            wp_chunk = wpl.tile([128, C], F32, tag="wp_chunk")
            nc.sync.dma_start(
                wp_chunk, wp_in.ap()[ks * 128:(ks + 1) * 128, :])
            nc.vector.tensor_copy(wp_bf[:, ks], wp_chunk)
    bp_sb = projp.tile([128, C], F32)
    nc.sync.dma_start(bp_sb, _bcast(bp_in.ap(), 128))

    # ---------- Phase 3: attention, software-pipelined over 4 stages ----------

    def emit_kext(i):
        b, h = SEQ[i]
        c0 = b * N
        hp = h * D
        kThx = att.tile([65, N], F16, tag="kThx", bufs=2, name="kThx")
        kTcross = att.tile([128, N], F16, tag="kTcross", bufs=2, name="kTcross")
        nc.vector.tensor_copy(kThx[0:64, :], kT_hi[hp:hp + D, c0:c0 + N])
        nc.gpsimd.memset(kThx[64:65, :], 1.0)
        nc.vector.tensor_copy(kTcross[0:64, :], kT_hi[hp:hp + D, c0:c0 + N])
        nc.vector.tensor_copy(kTcross[64:128, :], kT_lo[hp:hp + D, c0:c0 + N])
        kext_t[i] = (kThx, kTcross)

    def emit_T_j(i, j, fillers):
        b, h = SEQ[i]
        c0 = b * N
        hp = h * D
        kThx, kTcross = kext_t[i]
        biasT = biasT_t[i]
        if j == 0:
            av_t[i] = att.tile([65, 4, 512], F32, tag="av", bufs=2,
                               name=f"av{i}")
        qs = slice(c0 + j * 512, c0 + (j + 1) * 512)
        qhx = att.tile([65, 512], F16, tag="qhx", bufs=2, name="qhx")
        qcross = att.tile([128, 512], F16, tag="qcross", bufs=2, name="qcross")
        nc.vector.tensor_copy(qhx[0:64, :], qT_hi[hp:hp + D, qs])
        nc.vector.tensor_copy(qhx[64:65, :], biasT[0:1, j * 512:(j + 1) * 512])
        nc.vector.tensor_copy(qcross[0:64, :], qT_lo[hp:hp + D, qs])
        nc.vector.tensor_copy(qcross[64:128, :], qT_hi[hp:hp + D, qs])
        eT = eTp.tile([128, NKT, 512], F16, tag="eT", name="eT")
        for kt in range(NKT):
            psT = p3.tile([128, 512], F32, tag="psT", bufs=3, name="psT")
            kslc = slice(kt * 128, (kt + 1) * 128)
            # hi*hi + bias row, then both cross terms fused in one K=128 mm:
            # [k_hi;k_lo] . [q_lo;q_hi] = k_hi*q_lo + k_lo*q_hi
            nc.tensor.matmul(psT, kThx[:, kslc], qhx, start=True, stop=False)
            nc.tensor.matmul(psT, kTcross[:, kslc], qcross,
                             start=False, stop=True)
            nc.scalar.activation(eT[:, kt], psT, Exp)
            # sprinkle next-stage max-pass work between score groups so the
            # PE never stalls in a burst behind the DVE reduce queue
            if kt % 4 == 3 and fillers:
                fillers.pop(0)()
        pav = p3.tile([65, 512], F32, tag="pav", bufs=1, name="pav")
        for kt in range(NKT):
            nc.tensor.matmul(pav, v_sb[:, b * 16 + kt, h, :], eT[:, kt],
                             start=(kt == 0), stop=(kt == NKT - 1))
        nc.vector.tensor_copy(av_t[i][:, j], pav)

    def emit_norm(i):
        b, h = SEQ[i]
        c0 = b * N
        hp = h * D
        av = av_t.pop(i)
        # 1/s = exp(-ln s) on ACT (a DVE reciprocal on a single-partition row
        # costs ~3.3us), then broadcast across 64 partitions via a K=1 outer
        # product on the PE (which is idle at stage boundaries) so the gpsimd
        # queue stays empty for the collective doorbell
        rj0 = att.tile([1, 4, 512], F32, tag="rj0", bufs=2, name="rj0")
        nc.vector.tensor_copy(rj0, av[64:65, :, :])
        nc.scalar.activation(rj0, rj0, mybir.ActivationFunctionType.Ln)
        nc.scalar.activation(rj0, rj0, mybir.ActivationFunctionType.Exp,
                             scale=-1.0)
        for j in range(4):
            qs = slice(c0 + j * 512, c0 + (j + 1) * 512)
            rb = p3.tile([64, 512], F32, tag="pav", bufs=1, name="rb")
            nc.tensor.matmul(rb, ones64[0:1, :], rj0[0:1, j],
                             start=True, stop=True)
            nc.vector.tensor_mul(outT_sb[hp:hp + D, qs], av[0:64, j], rb)

    # pipeline: stage i's transposed pass interleaves stage i+1's max pass
    att = attctx.enter_context(tc.tile_pool(name="att", bufs=1))
    eTp = attctx.enter_context(tc.tile_pool(name="eTp", bufs=2))
    p3 = attctx.enter_context(tc.tile_pool(name="p3", bufs=1, space="PSUM"))
    emit_stats_flatten(0, p3, "pav")
    emit_kext(0)
    # per-batch AllToAll: batch b's shard j = its rows [j*256,(j+1)*256)
    # (core j's output = batch0 rows j*256.. plus batch1 rows j*256..)
    HRS = RS // 2  # 256 rows per batch per core
    a2a_in = [dram.tile([NCORES * 128, HRS], F16, name=f"a2ai{b}")
              for b in range(B)]
    a2a_out = [dram.tile([NCORES * 128, HRS], F16, name=f"a2ao{b}")
               for b in range(B)]
    lhsT_proj = [projp.tile([128, KS, HRS], F16, name=f"lhsTp{b}")
                 for b in range(B)]

    def emit_a2a(b):
        nc.sync.dma_start(
            a2a_in[b].rearrange("(j p) r -> p j r", j=NCORES),
            outT_sb[:, b * N:(b + 1) * N].rearrange("p (j r) -> p j r",
                                                    j=NCORES))
        nc.gpsimd.collective_compute(
            "AllToAll", mybir.AluOpType.bypass,
            replica_groups=[list(range(NCORES))],
            ins=[a2a_in[b][:]], outs=[a2a_out[b][:]])
        nc.sync.dma_start(
            lhsT_proj[b],
            a2a_out[b].rearrange("(j p) r -> p j r", j=NCORES))

    fill_by_stage = {
        0: [(lambda mt=mt: emit_nat_pair(1, 2, mt)) for mt in range(NQT)],
        1: [(lambda mt=mt: emit_nat_mt(3, mt, p3, 2, tag="psA"))
            for mt in range(NQT // 2)],
        2: [(lambda mt=mt: emit_nat_mt(3, mt, p3, 2, tag="psA"))
            for mt in range(NQT // 2, NQT)],
        3: [],
    }
    for i in range(4):
        fillers = fill_by_stage[i]
        for j in range(4):
            emit_T_j(i, j, fillers)
        while fillers:
            fillers.pop(0)()
        if i == 0:
            emit_stats_flatten(1, p3, "pav")
            emit_kext(1)
            emit_stats_flatten(2, p3, "pav")
        elif i == 1:
            emit_kext(2)
        elif i == 2:
            emit_stats_flatten(3, p3, "pav")
            emit_kext(3)
        emit_norm(i)
        if i == 1:
            emit_a2a(0)
        elif i == 3:
            emit_a2a(1)
    attctx.close()

    # ---------- Phase 4: output projection (batch-0 rows overlap A2A #2) ----
    with tc.tile_pool(name="proj", bufs=1) as proj, \
         tc.tile_pool(name="p4", bufs=1, space="PSUM") as p4:
        for m in range(RS // 128):
            lhsT_b = lhsT_proj[m // 2]
            mo = (m % 2) * 128
            for nt in range(C // 512):
                pp = p4.tile([128, 512], F32, tag="pp", bufs=4)
                for ks in range(KS):
                    nc.tensor.matmul(pp, lhsT_b[:, ks, mo:mo + 128],
                                     wp_bf[:, ks, nt * 512:(nt + 1) * 512],
                                     start=(ks == 0), stop=(ks == KS - 1))
                o_sb = proj.tile([128, 512], F32, tag="o_sb", bufs=4)
                nc.vector.tensor_add(o_sb, pp,
                                     bp_sb[:, nt * 512:(nt + 1) * 512])
                nc.sync.dma_start(
                    out_t.ap()[m * 128:(m + 1) * 128, nt * 512:(nt + 1) * 512],
                    o_sb)
    ctx.close()


_PROGRAM = None


def _get_program():
    global _PROGRAM
    if _PROGRAM is None:
        _PROGRAM = build_program()
    return _PROGRAM


def kernel(x, W_qkv, b_qkv, W_proj, b_proj, _trace=False):
    xT = np.ascontiguousarray(np.asarray(x, dtype=np.float32).reshape(R, C).T)
    W_qkv = np.asarray(W_qkv, dtype=np.float32)
    b_qkv = np.asarray(b_qkv, dtype=np.float32)
    W_proj = np.ascontiguousarray(np.asarray(W_proj, dtype=np.float32))
    b_proj = np.ascontiguousarray(np.asarray(b_proj, dtype=np.float32))

    in_maps = []
    for i in range(NCORES):
        lo = i * D2            # first column of my heads within a qkv block
        hi = lo + D2
        in_maps.append({
            "xT": xT,
            "wq": np.ascontiguousarray(W_qkv[:, 0 * C + lo:0 * C + hi]),
            "wk": np.ascontiguousarray(W_qkv[:, 1 * C + lo:1 * C + hi]),
            "wv": np.ascontiguousarray(W_qkv[:, 2 * C + lo:2 * C + hi]),
            "bq": np.ascontiguousarray(b_qkv[0 * C + lo:0 * C + hi]),
            "bk": np.ascontiguousarray(b_qkv[1 * C + lo:1 * C + hi]),
            "bv": np.ascontiguousarray(b_qkv[2 * C + lo:2 * C + hi]),
            "wp": W_proj,
            "bp": b_proj,
        })

    nc = _get_program()
    res = bass_utils.run_bass_kernel_spmd(
        nc, in_maps, core_ids=list(range(NCORES)), trace=_trace)
    out = np.empty((R, C), dtype=np.float32)
    HRS = RS // 2
    for i in range(NCORES):
        o = res.results[i]["out"]
        for b in range(B):
            out[b * N + i * HRS: b * N + (i + 1) * HRS] = \
                o[b * HRS:(b + 1) * HRS]
    if _trace:
        kernel.last_results = res
    return out.reshape(B, N, C)


# revision 29
# speedup vs baseline: 1.0835x; 1.0569x over previous
"""Trainium2 Bass kernel for 16-head MHA (b=2, n=2048, c=1024, d=64).

Reference semantics (note the inverted scale "bug" reproduced faithfully):
    qkv = x @ W_qkv + b_qkv
    scores = (q @ k^T) * sqrt(d)          # multiplied, not divided
    out = softmax(scores) @ v
    y = concat_heads(out) @ W_proj + b_proj

Sharding: tensor-parallel over heads. Each of the 8 cores computes QKV +
attention for its 2 heads (head-dim-transposed layouts so no activation
transposes are needed beyond one x^T pass), then a single AllToAll moves the
per-head attention outputs into a row-sharded layout and each core computes
the final projection for its 512-row output shard. Host concatenates shards.

Precision strategy: exactness is needed only upstream of exp (the *sqrt(d)
score scale amplifies absolute errors into exp-space relative errors).
Those matmuls use an fp16 hi/lo split, 3 accumulating passes
(hi*hi + hi*lo + lo*hi; the dropped lo*lo term is negligible) — products
are exact in the PE and accumulate in fp32. The row-max pass needs no
precision at all (softmax renormalization cancels any max offset exactly),
so it runs single-pass on the hi parts. exp scores, A@V and the projection
are plain fp16 (inputs exact to ~1e-3, no amplification).

Softmax plumbing: the transposed-score matmul gets an extra contraction row
(ones in k^T_ext, -rowmax in q_ext) so exp(psum) needs no separate bias op;
V gets an extra ones *column* so the A@V matmul also yields the softmax
denominators; one reciprocal + gpsimd partition-broadcast + multiply
normalizes each head-output block (emitted late so the DVE reciprocal never
blocks the PE-feeding copy chain).

The attention phase is software-pipelined at emission level: the row-max
pass of stage i+1 is interleaved with the transposed pass of stage i, so
the PE never drains while the stats chain (transpose/flatten/DMA) of the
next stage resolves.
"""

import sys
from contextlib import ExitStack

sys.path.insert(0, "/opt/trn_rl_repo")

import numpy as np

import concourse.bass as bass
import concourse.tile as tile
from concourse import bacc, mybir
from concourse import bass_utils
from concourse.masks import make_identity

# Problem shape (hardcoded per contract)
B, N, C = 2, 2048, 1024
H, D = 16, 64
NCORES = 8
HPC = H // NCORES          # heads per core = 2
D2 = HPC * D               # 128 = per-core slice of the concat dim
R = B * N                  # 4096 flattened rows
RS = R // NCORES           # 512 output rows per core
KS = C // 128              # 8 contraction blocks of 128
CHUNK = 512                # rows per x^T/qkv chunk
NCH = R // CHUNK           # 8 chunks
NQT = N // 128             # 16 query tiles per batch
NKT = N // 128             # 16 key tiles per batch
F32 = mybir.dt.float32
F16 = mybir.dt.float16

INV_SCALE = float(np.sqrt(D))  # 8.0, multiplied into q


def _bcast(ap, parts):
    """Broadcast a DRAM AP across `parts` partitions (step-0 partition dim)."""
    return bass.AP(tensor=ap.tensor, offset=ap.offset,
                   ap=[[0, parts]] + list(ap.ap))


def build_program():
    nc = bacc.Bacc("TRN2", target_bir_lowering=False, debug=False,
                   num_devices=NCORES)

    xT_in = nc.dram_tensor("xT", [C, R], F32, kind="ExternalInput")
    wq_in = nc.dram_tensor("wq", [C, D2], F32, kind="ExternalInput")
    wk_in = nc.dram_tensor("wk", [C, D2], F32, kind="ExternalInput")
    wv_in = nc.dram_tensor("wv", [C, D2], F32, kind="ExternalInput")
    bq_in = nc.dram_tensor("bq", [D2], F32, kind="ExternalInput")
    bk_in = nc.dram_tensor("bk", [D2], F32, kind="ExternalInput")
    bv_in = nc.dram_tensor("bv", [D2], F32, kind="ExternalInput")
    wp_in = nc.dram_tensor("wp", [C, C], F32, kind="ExternalInput")
    bp_in = nc.dram_tensor("bp", [C], F32, kind="ExternalInput")
    out_t = nc.dram_tensor("out", [RS, C], F32, kind="ExternalOutput")

    with tile.TileContext(nc) as tc:
        kernel_body(tc, xT_in, wq_in, wk_in, wv_in, bq_in, bk_in, bv_in,
                    wp_in, bp_in, out_t)
    nc.compile()
    return nc


def kernel_body(tc, xT_in, wq_in, wk_in, wv_in, bq_in, bk_in, bv_in,
                wp_in, bp_in, out_t):
    nc = tc.nc
    Exp = mybir.ActivationFunctionType.Exp
    Ident = mybir.ActivationFunctionType.Identity

    ctx = ExitStack()
    consts = ctx.enter_context(tc.tile_pool(name="consts", bufs=1))
    persist = ctx.enter_context(tc.tile_pool(name="persist", bufs=1))
    dram = ctx.enter_context(tc.tile_pool(name="dram", bufs=1, space="DRAM"))

    ident = consts.tile([128, 128], F32)
    make_identity(nc, ident)
    ones64 = consts.tile([1, 64], F16)
    nc.vector.memset(ones64, 1.0)

    # --- weights / biases for qkv (hi/lo split in fp16) ---
    with tc.tile_pool(name="wstage", bufs=2) as wstage:
        def split_w(name, t_in):
            w_f32 = wstage.tile([128, KS, D2], F32, tag="w_f32", name=name)
            nc.sync.dma_start(w_f32,
                              t_in.ap().rearrange("(ks p) m -> p ks m", p=128))
            hi = consts.tile([128, KS, D2], F16, name=name + "_hi")
            lo = consts.tile([128, KS, D2], F16, name=name + "_lo")
            nc.vector.tensor_copy(hi, w_f32)
            nc.vector.tensor_sub(lo, w_f32, hi)
            return hi, lo

        wq_hi, wq_lo = split_w("wq", wq_in)
        wk_hi, wk_lo = split_w("wk", wk_in)
        wv_f32 = wstage.tile([128, KS, D2], F32, tag="w_f32", name="wv")
        nc.sync.dma_start(wv_f32, wv_in.ap().rearrange("(ks p) m -> p ks m", p=128))
        wv_bf = consts.tile([128, KS, D2], F16)
        nc.vector.tensor_copy(wv_bf, wv_f32)

    bq_sb = consts.tile([128, 1], F32)
    bk_sb = consts.tile([128, 1], F32)
    nc.sync.dma_start(bq_sb, bq_in.ap().rearrange("(p o) -> p o", o=1))
    nc.sync.dma_start(bk_sb, bk_in.ap().rearrange("(p o) -> p o", o=1))
    bq8_sb = consts.tile([128, 1], F32)
    nc.scalar.mul(bq8_sb, bq_sb, INV_SCALE)
    bv_sb = consts.tile([128, D2], F32)
    nc.sync.dma_start(bv_sb, _bcast(bv_in.ap(), 128))

    # --- persistent activations (fp16) ---
    qT_hi = persist.tile([128, R], F16)   # sqrt(d) * q^T, hi part
    qT_lo = persist.tile([128, R], F16)
    kT_hi = persist.tile([128, R], F16)
    kT_lo = persist.tile([128, R], F16)
    # v with a ones column per head: [p, row_tile, head, 65]
    v_sb = persist.tile([128, R // 128, HPC, D + 1], F16)
    nc.vector.memset(v_sb[:, :, :, D:D + 1], 1.0)
    outT_sb = persist.tile([128, R], F16)

    SEQ = [(b, h) for b in range(B) for h in range(HPC)]
    attctx = ExitStack()
    attE = ctx.enter_context(tc.tile_pool(name="attE", bufs=1))

    stats_t = {}   # stage -> stats tile [128, 16] (negated row maxes)
    biasT_t = {}   # stage -> [1, N] flattened -rowmax
    kext_t = {}    # stage -> (kThx, kTlx)
    av_t = {}      # stage -> [65, 4, 512] unnormalized head outputs + sums

    def emit_nat_mt(i, mt, pool, ps_bufs, tag="ps"):
        b, h = SEQ[i]
        c0 = b * N
        hp = h * D
        if mt == 0:
            stats_t[i] = attE.tile([128, 16], F32, tag="stats", bufs=2,
                                  name=f"stats{i}")
        stats = stats_t[i]
        q_l = qT_hi[hp:hp + D, c0 + mt * 128: c0 + (mt + 1) * 128]
        mx = attE.tile([128, 4], F32, tag="mx" + tag, bufs=2, name="mx")
        for j in range(4):
            p = pool.tile([128, 512], F32, tag=tag, bufs=ps_bufs, name="ps")
            nc.tensor.matmul(
                p, q_l,
                kT_hi[hp:hp + D, c0 + j * 512:c0 + (j + 1) * 512],
                start=True, stop=True)
            nc.vector.reduce_max(mx[:, j:j + 1], p,
                                 axis=mybir.AxisListType.X)
        nc.vector.reduce_max(stats[:, mt:mt + 1], mx,
                             axis=mybir.AxisListType.X, negate=True)

    def emit_nat_pair(ia, ib, mt):
        # two heads on disjoint PE row groups -> the 8 matmuls run pairwise
        # concurrently in the array
        ba, ha = SEQ[ia]
        bb, hb = SEQ[ib]
        assert ha != hb
        for i, tag in ((ia, "psA"), (ib, "psB")):
            if mt == 0:
                stats_t[i] = attE.tile([128, 16], F32, tag="stats", bufs=2,
                                       name=f"stats{i}")
        mxs = {}
        for i, tag in ((ia, "psA"), (ib, "psB")):
            mxs[i] = attE.tile([128, 4], F32, tag="mx" + tag, bufs=2, name="mx")
        ps = {}
        for j in range(4):
            for i, tag in ((ia, "psA"), (ib, "psB")):
                b, h = SEQ[i]
                c0 = b * N
                hp = h * D
                q_l = qT_hi[hp:hp + D, c0 + mt * 128: c0 + (mt + 1) * 128]
                p = p3.tile([128, 512], F32, tag=tag, bufs=2, name="ps")
                nc.tensor.matmul(
                    p, q_l,
                    kT_hi[hp:hp + D, c0 + j * 512:c0 + (j + 1) * 512],
                    start=True, stop=True)
                ps[i] = p
            for i, tag in ((ia, "psA"), (ib, "psB")):
                nc.vector.reduce_max(mxs[i][:, j:j + 1], ps[i],
                                     axis=mybir.AxisListType.X)
        for i, tag in ((ia, "psA"), (ib, "psB")):
            nc.vector.reduce_max(stats_t[i][:, mt:mt + 1], mxs[i],
                                 axis=mybir.AxisListType.X, negate=True)

    def emit_stats_flatten(i, pool, tag):
        stats = stats_t.pop(i)
        pstat = pool.tile([16, 128], F32, tag=tag, bufs=1, name="pstat")
        nc.tensor.transpose(pstat, stats, ident)
        statsT = attE.tile([16, 128], F16, tag="statsT", bufs=2, name="statsT")
        nc.vector.tensor_copy(statsT, pstat)
        biasT = attE.tile([1, N], F16, tag="biasT", bufs=2, name="biasT")
        nc.sync.dma_start(biasT.rearrange("s (m q) -> s m q", m=16), statsT)
        biasT_t[i] = biasT

    # ---------- Phase 1+2: x^T chunks and QKV projections ----------
    xT_view = xT_in.ap().rearrange("(ks p) r -> p ks r", p=128)
    with tc.tile_pool(name="xload", bufs=2) as xload, \
         tc.tile_pool(name="p12", bufs=1, space="PSUM") as p12:
        for ch in range(NCH):
            r0 = ch * CHUNK
            xT = xload.tile([128, KS, CHUNK], F32, tag="xT")
            xT_hi = xload.tile([128, KS, CHUNK], F16, tag="xT_hi")
            xT_lo = xload.tile([128, KS, CHUNK], F16, tag="xT_lo")
            # fp16 hi/lo split (hi cast on ACT, lo residual on DVE),
            # in pieces so compute starts before the full chunk lands
            npc = 4 if ch == 0 else 2
            for hf in range(npc):
                ksl = slice(hf * KS // npc, (hf + 1) * KS // npc)
                nc.sync.dma_start(xT[:, ksl], xT_view[:, ksl, r0:r0 + CHUNK])
                nc.scalar.copy(xT_hi[:, ksl], xT[:, ksl])
                nc.vector.tensor_sub(xT_lo[:, ksl], xT[:, ksl], xT_hi[:, ksl])
            # q^T, k^T for this chunk (128 rows = 2 heads * 64), 3-pass split
            for (whi, wlo, dst_hi, dst_lo, bias, scale) in (
                    (wq_hi, wq_lo, qT_hi, qT_lo, bq8_sb, INV_SCALE),
                    (wk_hi, wk_lo, kT_hi, kT_lo, bk_sb, 1.0)):
                pqk = p12.tile([128, CHUNK], F32, tag="pqk", bufs=2)
                passes = [(whi, xT_hi), (whi, xT_lo), (wlo, xT_hi)]
                for pi, (w_p, x_p) in enumerate(passes):
                    for ks in range(KS):
                        nc.tensor.matmul(pqk, w_p[:, ks], x_p[:, ks],
                                         start=(pi == 0 and ks == 0),
                                         stop=(pi == 2 and ks == KS - 1))
                # hi = f16(scale*psum + bias); lo = f16((scale*psum+bias) - hi)
                nc.scalar.activation(dst_hi[:, r0:r0 + CHUNK], pqk, Ident,
                                     bias=bias, scale=scale)
                tmp = xload.tile([128, CHUNK], F32, tag="qk_tmp", bufs=2)
                nc.vector.tensor_scalar(tmp, pqk, scalar1=scale, scalar2=bias,
                                        op0=mybir.AluOpType.mult,
                                        op1=mybir.AluOpType.add)
                nc.gpsimd.tensor_sub(dst_lo[:, r0:r0 + CHUNK], tmp,
                                     dst_hi[:, r0:r0 + CHUNK])
            # v natural layout (single-pass fp16)
            for m in range(4):
                pv = p12.tile([128, D2], F32, tag="pv", bufs=1)
                for ks in range(KS):
                    nc.tensor.matmul(pv, xT_hi[:, ks, m * 128:(m + 1) * 128],
                                     wv_bf[:, ks],
                                     start=(ks == 0), stop=(ks == KS - 1))
                for h in range(HPC):
                    nc.vector.tensor_add(
                        v_sb[:, ch * 4 + m, h, 0:D],
                        pv[:, h * D:(h + 1) * D],
                        bv_sb[:, h * D:(h + 1) * D])
                del pv
                # batch-0 q/k ready after chunk 3: overlap stage-0 max pass
                if ch >= NCH // 2:
                    emit_nat_mt(0, (ch - NCH // 2) * 4 + m, p12, 2)

    # projection weights: load + cast early so the post-A2A path is short
    projp = ctx.enter_context(tc.tile_pool(name="projp", bufs=1))
    wp_bf = projp.tile([128, KS, C], F16)
    with tc.tile_pool(name="wpl", bufs=2) as wpl:
        for ks in range(KS):
            wp_chunk = wpl.tile([128, C], F32, tag="wp_chunk")
            nc.sync.dma_start(
                wp_chunk, wp_in.ap()[ks * 128:(ks + 1) * 128, :])
            nc.vector.tensor_copy(wp_bf[:, ks], wp_chunk)
    bp_sb = projp.tile([128, C], F32)
    nc.sync.dma_start(bp_sb, _bcast(bp_in.ap(), 128))

    # ---------- Phase 3: attention, software-pipelined over 4 stages ----------

    def emit_kext(i):
        b, h = SEQ[i]
        c0 = b * N
        hp = h * D
        kThx = att.tile([65, N], F16, tag="kThx", bufs=2, name="kThx")
        kTcross = att.tile([128, N], F16, tag="kTcross", bufs=2, name="kTcross")
        nc.vector.tensor_copy(kThx[0:64, :], kT_hi[hp:hp + D, c0:c0 + N])
        nc.gpsimd.memset(kThx[64:65, :], 1.0)
        nc.vector.tensor_copy(kTcross[0:64, :], kT_hi[hp:hp + D, c0:c0 + N])
        nc.vector.tensor_copy(kTcross[64:128, :], kT_lo[hp:hp + D, c0:c0 + N])
        kext_t[i] = (kThx, kTcross)

    def emit_T_j(i, j, fillers):
        b, h = SEQ[i]
        c0 = b * N
        hp = h * D
        kThx, kTcross = kext_t[i]
        biasT = biasT_t[i]
        if j == 0:
            av_t[i] = att.tile([65, 4, 512], F32, tag="av", bufs=2,
                               name=f"av{i}")
        qs = slice(c0 + j * 512, c0 + (j + 1) * 512)
        qhx = att.tile([65, 512], F16, tag="qhx", bufs=2, name="qhx")
        qcross = att.tile([128, 512], F16, tag="qcross", bufs=2, name="qcross")
        nc.vector.tensor_copy(qhx[0:64, :], qT_hi[hp:hp + D, qs])
        nc.vector.tensor_copy(qhx[64:65, :], biasT[0:1, j * 512:(j + 1) * 512])
        nc.vector.tensor_copy(qcross[0:64, :], qT_lo[hp:hp + D, qs])
        nc.vector.tensor_copy(qcross[64:128, :], qT_hi[hp:hp + D, qs])
        eT = eTp.tile([128, NKT, 512], F16, tag="eT", name="eT")
        for kt in range(NKT):
            psT = p3.tile([128, 512], F32, tag="psT", bufs=3, name="psT")
            kslc = slice(kt * 128, (kt + 1) * 128)
            # hi*hi + bias row, then both cross terms fused in one K=128 mm:
            # [k_hi;k_lo] . [q_lo;q_hi] = k_hi*q_lo + k_lo*q_hi
            nc.tensor.matmul(psT, kThx[:, kslc], qhx, start=True, stop=False)
            nc.tensor.matmul(psT, kTcross[:, kslc], qcross,
                             start=False, stop=True)
            nc.scalar.activation(eT[:, kt], psT, Exp)
            # sprinkle next-stage max-pass work between score groups so the
            # PE never stalls in a burst behind the DVE reduce queue
            if kt % 4 == 3 and fillers:
                fillers.pop(0)()
        pav = p3.tile([65, 512], F32, tag="pav", bufs=1, name="pav")
        for kt in range(NKT):
            nc.tensor.matmul(pav, v_sb[:, b * 16 + kt, h, :], eT[:, kt],
                             start=(kt == 0), stop=(kt == NKT - 1))
        nc.vector.tensor_copy(av_t[i][:, j], pav)

    def emit_norm(i):
        b, h = SEQ[i]
        c0 = b * N
        hp = h * D
        av = av_t.pop(i)
        # 1/s = exp(-ln s) on ACT (a DVE reciprocal on a single-partition row
        # costs ~3.3us), then broadcast across 64 partitions via a K=1 outer
        # product on the PE (which is idle at stage boundaries) so the gpsimd
        # queue stays empty for the collective doorbell
        rj0 = att.tile([1, 4, 512], F32, tag="rj0", bufs=2, name="rj0")
        nc.vector.tensor_copy(rj0, av[64:65, :, :])
        nc.scalar.activation(rj0, rj0, mybir.ActivationFunctionType.Ln)
        rj16 = att.tile([1, 4, 512], F16, tag="rj16", bufs=2, name="rj16")
        nc.scalar.activation(rj16, rj0, mybir.ActivationFunctionType.Exp,
                             scale=-1.0)
        for j in range(4):
            qs = slice(c0 + j * 512, c0 + (j + 1) * 512)
            rb = p3.tile([64, 512], F32, tag="pav", bufs=1, name="rb")
            nc.tensor.matmul(rb, ones64[0:1, :], rj16[0:1, j],
                             start=True, stop=True)
            nc.vector.tensor_mul(outT_sb[hp:hp + D, qs], av[0:64, j], rb)

    # pipeline: stage i's transposed pass interleaves stage i+1's max pass
    att = attctx.enter_context(tc.tile_pool(name="att", bufs=1))
    eTp = attctx.enter_context(tc.tile_pool(name="eTp", bufs=2))
    p3 = attctx.enter_context(tc.tile_pool(name="p3", bufs=1, space="PSUM"))
    emit_stats_flatten(0, p3, "pav")
    emit_kext(0)
    # per-batch AllToAll: batch b's shard j = its rows [j*256,(j+1)*256)
    # (core j's output = batch0 rows j*256.. plus batch1 rows j*256..)
    HRS = RS // 2  # 256 rows per batch per core
    a2a_in = [dram.tile([NCORES * 128, HRS], F16, name=f"a2ai{b}")
              for b in range(B)]
    a2a_out = [dram.tile([NCORES * 128, HRS], F16, name=f"a2ao{b}")
               for b in range(B)]
    lhsT_proj = [projp.tile([128, KS, HRS], F16, name=f"lhsTp{b}")
                 for b in range(B)]

    def emit_a2a(b):
        nc.sync.dma_start(
            a2a_in[b].rearrange("(j p) r -> p j r", j=NCORES),
            outT_sb[:, b * N:(b + 1) * N].rearrange("p (j r) -> p j r",
                                                    j=NCORES))
        nc.gpsimd.collective_compute(
            "AllToAll", mybir.AluOpType.bypass,
            replica_groups=[list(range(NCORES))],
            ins=[a2a_in[b][:]], outs=[a2a_out[b][:]])
        nc.sync.dma_start(
            lhsT_proj[b],
            a2a_out[b].rearrange("(j p) r -> p j r", j=NCORES))

    fill_by_stage = {
        0: [(lambda mt=mt: emit_nat_pair(1, 2, mt)) for mt in range(NQT)],
        1: [(lambda mt=mt: emit_nat_mt(3, mt, p3, 2, tag="psA"))
            for mt in range(NQT // 2)],
        2: [(lambda mt=mt: emit_nat_mt(3, mt, p3, 2, tag="psA"))
            for mt in range(NQT // 2, NQT)],
        3: [],
    }
    def boundary(i):
        if i == 0:
            emit_stats_flatten(1, p3, "pav")
            emit_kext(1)
            emit_stats_flatten(2, p3, "pav")
        elif i == 1:
            emit_kext(2)
        elif i == 2:
            emit_stats_flatten(3, p3, "pav")
            emit_kext(3)

    for i in range(4):
        fillers = fill_by_stage[i]
        for j in range(4):
            emit_T_j(i, j, fillers)
            if j == 2:
                while fillers:
                    fillers.pop(0)()
                boundary(i)
        emit_norm(i)
        if i == 1:
            emit_a2a(0)
        elif i == 3:
            emit_a2a(1)
    attctx.close()

    # ---------- Phase 4: output projection (batch-0 rows overlap A2A #2) ----
    with tc.tile_pool(name="proj", bufs=1) as proj, \
         tc.tile_pool(name="p4", bufs=1, space="PSUM") as p4:
        for m in range(RS // 128):
            lhsT_b = lhsT_proj[m // 2]
            mo = (m % 2) * 128
            for nt in range(C // 512):
                pp = p4.tile([128, 512], F32, tag="pp", bufs=4)
                for ks in range(KS):
                    nc.tensor.matmul(pp, lhsT_b[:, ks, mo:mo + 128],
                                     wp_bf[:, ks, nt * 512:(nt + 1) * 512],
                                     start=(ks == 0), stop=(ks == KS - 1))
                o_sb = proj.tile([128, 512], F32, tag="o_sb", bufs=4)
                nc.vector.tensor_add(o_sb, pp,
                                     bp_sb[:, nt * 512:(nt + 1) * 512])
                nc.sync.dma_start(
                    out_t.ap()[m * 128:(m + 1) * 128, nt * 512:(nt + 1) * 512],
                    o_sb)
    ctx.close()


_PROGRAM = None


def _get_program():
    global _PROGRAM
    if _PROGRAM is None:
        _PROGRAM = build_program()
    return _PROGRAM


def kernel(x, W_qkv, b_qkv, W_proj, b_proj, _trace=False):
    xT = np.ascontiguousarray(np.asarray(x, dtype=np.float32).reshape(R, C).T)
    W_qkv = np.asarray(W_qkv, dtype=np.float32)
    b_qkv = np.asarray(b_qkv, dtype=np.float32)
    W_proj = np.ascontiguousarray(np.asarray(W_proj, dtype=np.float32))
    b_proj = np.ascontiguousarray(np.asarray(b_proj, dtype=np.float32))

    in_maps = []
    for i in range(NCORES):
        lo = i * D2            # first column of my heads within a qkv block
        hi = lo + D2
        in_maps.append({
            "xT": xT,
            "wq": np.ascontiguousarray(W_qkv[:, 0 * C + lo:0 * C + hi]),
            "wk": np.ascontiguousarray(W_qkv[:, 1 * C + lo:1 * C + hi]),
            "wv": np.ascontiguousarray(W_qkv[:, 2 * C + lo:2 * C + hi]),
            "bq": np.ascontiguousarray(b_qkv[0 * C + lo:0 * C + hi]),
            "bk": np.ascontiguousarray(b_qkv[1 * C + lo:1 * C + hi]),
            "bv": np.ascontiguousarray(b_qkv[2 * C + lo:2 * C + hi]),
            "wp": W_proj,
            "bp": b_proj,
        })

    nc = _get_program()
    res = bass_utils.run_bass_kernel_spmd(
        nc, in_maps, core_ids=list(range(NCORES)), trace=_trace)
    out = np.empty((R, C), dtype=np.float32)
    HRS = RS // 2
    for i in range(NCORES):
        o = res.results[i]["out"]
        for b in range(B):
            out[b * N + i * HRS: b * N + (i + 1) * HRS] = \
                o[b * HRS:(b + 1) * HRS]
    if _trace:
        kernel.last_results = res
    return out.reshape(B, N, C)
